# revision 32
# baseline (speedup 1.0000x reference)
"""GAT (3 parallel attention heads + FC classifier) on 8 Trainium2 NeuronCores.

Sharding: data-parallel over the batch (graph) dim — 2 graphs per core, layer
weights replicated. One SPMD Bass program; per-core inputs are the graph
slices.

Math (per graph, per layer l):
    h = x @ W_l                                  [N, H]
    f1 = h @ a1_l, f2 = h @ a2_l                 [N]
    e_ij = leaky_relu(f1_i + f2_j, 0.2)
    att = softmax_j(where(adj_ij > 0, e, -inf))
    out = relu(att @ h)
Key identity used on-device (lrelu -> max of two exponentials):
    exp(lrelu(t)) = max(exp(t), exp(0.2 t)),  t = f1_i + f2_j
so with p=exp(f1), r=exp(0.2 f1) (free-dim broadcast tiles) and q=exp(f2),
s=exp(0.2 f2) (per-partition scalars), the masked unnormalized attention is
    T_ji = adj_ij * max(p_i * q_j, r_i * s_j)
built in transposed (neighbor-j on partitions) layout so the TensorE can
contract over j directly:  numer/denom = T^T @ [h | 1].
Then out_i = relu(numer_i)/denom_i, mean-pool over nodes folded into the
pooling matmul (rhs = 1/denom column), then FC1+relu, FC2, softmax.
"""
import numpy as np

import concourse.bass as bass
from concourse.masks import make_identity
import concourse.tile as tile
import concourse.mybir as mybir
from concourse.bass_utils import run_bass_kernel_spmd
from concourse.tile_rust import add_dep_helper

f32 = mybir.dt.float32
bf16 = mybir.dt.bfloat16
ALU = mybir.AluOpType
ACTF = mybir.ActivationFunctionType

B, N, F, H, L = 16, 1024, 128, 64, 3
FC, C = 128, 10
NCORES = 8
G = B // NCORES          # graphs per core
NCH = N // 128           # 8 node chunks
LEAK = 0.2
NBF = 973                # bf16-cast columns of the packed weight tile
WCOLS = 984


def _split_multi_waits(nc):
    """The cayman ISA structs have exactly one embedded sync-wait slot and
    this walrus build refuses instructions with more; split extras into
    preceding single-wait NoOp carriers on the same engine."""
    n = 0
    for fn in nc.m.functions:
        for blk in fn.blocks:
            out = []
            for inst in blk.instructions:
                si = inst.sync_info
                if si is not None and si.on_wait and len(si.on_wait) > 1:
                    waits = list(si.on_wait)
                    for w in waits[1:]:
                        out.append(mybir.InstNoOp(
                            name=f"{inst.name}_wc{n}", ins=[], outs=[],
                            engine=inst.engine,
                            sync_info=mybir.SyncInfo(on_wait=[w], on_update=[]),
                            bass_nofuse=True))
                        n += 1
                    si.on_wait = waits[:1]
                out.append(inst)
            blk.instructions = out
    return n


def build():
    nc = bass.Bass()

    x_d = nc.dram_tensor("x", [G, N, F], f32, kind="ExternalInput")
    adj_d = nc.dram_tensor("adj", [G, N, N], f32, kind="ExternalInput")
    wp_d = nc.dram_tensor("wpack", [128, WCOLS], f32, kind="ExternalInput")
    out_d = nc.dram_tensor("out", [G, C], f32, kind="ExternalOutput")

    # DRAM scratch: bf16 copies so the 2-byte-only xbar DMA-transpose applies.
    # One tensor per (graph, column-block): Tile tracks DRAM deps per-tensor,
    # so separate tensors keep the cast->transpose pipeline deps exact.
    adjb_scr = [[nc.dram_tensor(f"adjb_{g}_{b}", [N, 256], bf16, kind="Internal")
                 for b in range(4)] for g in range(G)]
    xb_scr = [nc.dram_tensor(f"xb_{g}", [N, F], bf16, kind="Internal")
              for g in range(G)]

    with tile.TileContext(nc) as tc:
        _build_body(nc, tc, x_d, adj_d, wp_d, out_d, adjb_scr, xb_scr)
    _split_multi_waits(nc)
    return nc


def _build_body(nc, tc, x_d, adj_d, wp_d, out_d, adjb_scr, xb_scr):
    from contextlib import ExitStack
    with ExitStack() as ctx:
        ep = ctx.enter_context

        consts = ep(tc.tile_pool(name="consts", bufs=1))
        p_adjT = ep(tc.tile_pool(name="adjT", bufs=2))
        p_xT = ep(tc.tile_pool(name="xT", bufs=2))
        p_bc = ep(tc.tile_pool(name="bc", bufs=3))      # p_b / r_b broadcasts
        p_h = ep(tc.tile_pool(name="h", bufs=3))        # haug / hTb / scaled
        p_w = ep(tc.tile_pool(name="w", bufs=4))        # v / m / T work tiles
        p_sm = ep(tc.tile_pool(name="sm", bufs=3))      # small vectors
        # PSUM: out accumulators 2x1 bank; wide 2x2 banks; small 2x1 bank
        ps_out = ep(tc.tile_pool(name="ps_out", bufs=4, space="PSUM"))
        ps_wide = ep(tc.tile_pool(name="ps_wide", bufs=1, space="PSUM"))
        ps_sm = ep(tc.tile_pool(name="ps_sm", bufs=2, space="PSUM"))
        ps_fc = ep(tc.tile_pool(name="ps_fc", bufs=1, space="PSUM"))

        # ---- identity for PE transposes, generated on-chip early ----
        ident = consts.tile([128, 128], f32)
        with tc.high_priority():
            make_identity(nc, ident)
        # ---- packed weights (host-packed, see kernel()) ----
        wp32 = consts.tile([128, WCOLS], f32)
        nc.sync.dma_start(out=wp32, in_=wp_d[:, :])
        wpb = consts.tile([128, NBF], bf16)
        nc.scalar.copy(wpb, wp32[:, 0:NBF])
        wb = wpb[:, 0:192].rearrange("p (l h) -> p l h", l=L)
        w1b = wpb[0:64, 192:576].rearrange("p (l f) -> p l f", l=L)
        w2b = wpb[:, 576:586]
        wa2c = wpb[:, 586:589]
        wa1m = wpb[:, 589:973].rearrange("p (l m) -> p l m", l=L)
        b1c = wp32[:, 973:974]
        b2r = wp32[0:1, 974:984]

        last_xpose = None
        xTs = []
        for g in range(G):
            # x: load natural, transpose on PE, cast on ACT (both graphs up
            # front: cheap, and the PE is idle during the adj startup)
            from contextlib import nullcontext
            with tc.high_priority() if g == 0 else nullcontext():
                x_sb = p_xT.tile([128, NCH, F], f32, tag="x_sb")
                nc.sync.dma_start(
                    out=x_sb, in_=x_d[g].rearrange("(c p) f -> p c f", p=128))
                xT = p_xT.tile([F, N], bf16, tag="xT")    # [feat, node]
                for half in range(2):
                    xt_ps = ps_wide.tile([128, 512], f32, tag="wide")
                    for cc in range(4):
                        nc.tensor.transpose(xt_ps[:, cc * 128:(cc + 1) * 128],
                                            x_sb[:, 4 * half + cc, :], ident)
                    nc.scalar.copy(xT[:, half * 512:half * 512 + 512], xt_ps)
            xTs.append(xT)
        for g in range(G):
            xT = xTs[g]
            adjT = p_adjT.tile([128, NCH, N], bf16)   # [j%128, j//128, i]
            for blk in range(4):
                ci = nc.gpsimd.dma_start(out=adjb_scr[g][blk][:, :],
                                         in_=adj_d[g, :, blk * 256:(blk + 1) * 256])
                if g > 0 and last_xpose is not None:
                    # keep graph-1 casts off the DMA fabric until graph-0's
                    # transposes (critical path) are through
                    add_dep_helper(ci.ins, last_xpose.ins, sync=True,
                                   reason="stagger g1 casts")
                for h2 in range(2):
                    xp = nc.sync.dma_start_transpose(
                        out=adjT[:, 2 * blk + h2, :],
                        in_=adjb_scr[g][blk][:, h2 * 128:(h2 + 1) * 128])
            last_xpose = xp

            pcol_ps = ps_fc.tile([H, L], f32, tag="fc")

            def prep(l):
                # f1 broadcast straight from xT: f1 = (W a1) . x, with the
                # W a1 product host-packed replicated as wa1m. The
                # r=exp(LEAK*f1) factor is row-constant and cancels in the
                # softmax ratio, so max(p q, r s) -> r * max(z q, s), r dropped
                z_bt = p_bc.tile([128, N], bf16, tag="z_b")
                for k in range(2):
                    f1bc_ps = ps_wide.tile([128, 512], f32, tag="wide")
                    nc.tensor.matmul(f1bc_ps, wa1m[:, l, :],
                                     xT[:, k * 512:(k + 1) * 512],
                                     start=True, stop=True)
                    nc.scalar.activation(z_bt[:, k * 512:(k + 1) * 512],
                                         f1bc_ps, ACTF.Exp, scale=1.0 - LEAK)
                # f2 per-partition columns: f2 = (W a2) . x
                f2c_ps = ps_sm.tile([128, NCH], f32, tag="small")
                for c in range(NCH):
                    nc.tensor.matmul(f2c_ps[:, c:c + 1],
                                     xT[:, c * 128:(c + 1) * 128],
                                     wa2c[:, l:l + 1], start=True, stop=True)
                q_all = p_sm.tile([128, NCH], f32, tag="q_all")
                nc.scalar.activation(q_all, f2c_ps, ACTF.Exp)
                s_all = p_sm.tile([128, NCH], f32, tag="s_all")
                nc.scalar.activation(s_all, f2c_ps, ACTF.Exp, scale=LEAK)
                # h natural chunks -> haug = [h | 1] (bf16)
                h_ps = ps_sm.tile([128, NCH, H], f32, tag="small")
                for c in range(NCH):
                    nc.tensor.matmul(h_ps[:, c, :],
                                     xT[:, c * 128:(c + 1) * 128], wb[:, l, :],
                                     start=True, stop=True)
                haug = p_h.tile([128, NCH, H + 1], bf16, tag="haug")
                nc.scalar.copy(haug[:, :, 0:H], h_ps)
                nc.vector.memset(haug[:, :, H:H + 1], 1.0)
                oA = ps_out.tile([128, 4, H + 1], f32, tag="out")
                oB = ps_out.tile([128, 4, H + 1], f32, tag="out")
                return z_bt, q_all, s_all, haug, oA, oB

            def chunk(st, c, first, last, split=False):
                z_bt, q_all, s_all, haug, oA, oB = st
                gm = p_w.tile([128, N], bf16, tag="gm")
                t = p_w.tile([128, N], bf16, tag="t")
                # split=True halves the ops so work can start on the first
                # half of z_bt before the second is ready (startup only)
                for lo, hi in (((0, 512), (512, 1024)) if split
                               else ((0, 1024),)):
                    nc.vector.tensor_scalar(
                        out=gm[:, lo:hi], in0=z_bt[:, lo:hi],
                        scalar1=q_all[:, c:c + 1], scalar2=s_all[:, c:c + 1],
                        op0=ALU.mult, op1=ALU.max)
                    nc.vector.tensor_mul(t[:, lo:hi], gm[:, lo:hi],
                                         adjT[:, c, lo:hi])
                for ib in range(NCH):
                    o = oA if ib < 4 else oB
                    nc.tensor.matmul(o[:, ib % 4, :],
                                     t[:, ib * 128:(ib + 1) * 128],
                                     haug[:, c, :], start=first, stop=last)

            def post(st, l):
                _, _, _, _, oA, oB = st
                rcp = p_sm.tile([128, NCH], f32, tag="rcp")
                nc.vector.reciprocal(rcp[:, 0:4], oA[:, :, H])
                nc.vector.reciprocal(rcp[:, 4:8], oB[:, :, H])
                rcpb = p_sm.tile([128, NCH], bf16, tag="rcpb")
                nc.scalar.copy(rcpb, rcp)
                scaled = p_h.tile([128, NCH, H], bf16, tag="scaled")
                nc.scalar.activation(scaled[:, 0:4, :], oA[:, :, 0:H], ACTF.Relu)
                nc.scalar.activation(scaled[:, 4:8, :], oB[:, :, 0:H], ACTF.Relu)
                for ib in range(NCH):
                    nc.tensor.matmul(pcol_ps[:, l:l + 1], scaled[:, ib, :],
                                     rcpb[:, ib:ib + 1],
                                     start=(ib == 0), stop=(ib == NCH - 1))

            # layers 0+1 interleaved per chunk (keeps the DVE fed while the
            # adjT transposes stream in), layer 2 after with adjT resident
            if g == 0:
                with tc.high_priority():
                    st0 = prep(0)
            else:
                st0 = prep(0)
            st1 = prep(1)
            for c in range(NCH):
                chunk(st0, c, c == 0, c == NCH - 1)
                chunk(st1, c, c == 0, c == NCH - 1)
            post(st0, 0)
            post(st1, 1)
            st2 = prep(2)
            for c in range(NCH):
                chunk(st2, c, c == 0, c == NCH - 1)
            post(st2, 2)

            # ---- classifier head ----
            pcolb = p_sm.tile([H, L], bf16, tag="pcolb")
            nc.scalar.copy(pcolb, pcol_ps)
            z_ps = ps_fc.tile([FC, 1], f32, tag="fc")
            for l in range(L):
                nc.tensor.matmul(z_ps, w1b[:, l, :], pcolb[:, l:l + 1],
                                 start=(l == 0), stop=(l == L - 1))
            zr = p_sm.tile([FC, 1], bf16, tag="zr")
            nc.scalar.activation(zr, z_ps, ACTF.Relu, bias=b1c, scale=1.0 / N)
            lg_ps = ps_fc.tile([1, C], f32, tag="fc")
            nc.tensor.matmul(lg_ps, zr, w2b, start=True, stop=True)
            lg = p_sm.tile([1, C], f32, tag="lg")
            nc.vector.tensor_add(lg, lg_ps, b2r)
            mx = p_sm.tile([1, 1], f32, tag="mx")
            nc.vector.reduce_max(mx, lg, axis=mybir.AxisListType.X)
            nmx = p_sm.tile([1, 1], f32, tag="nmx")
            nc.vector.tensor_scalar_mul(nmx, mx, -1.0)
            e = p_sm.tile([1, C], f32, tag="e")
            se = p_sm.tile([1, 1], f32, tag="se")
            nc.scalar.activation(e, lg, ACTF.Exp, bias=nmx, accum_out=se)
            rse = p_sm.tile([1, 1], f32, tag="rse")
            nc.vector.reciprocal(rse, se)
            prob = p_sm.tile([1, C], f32, tag="prob")
            nc.vector.tensor_scalar_mul(prob, e, rse[0:1, 0:1])
            nc.sync.dma_start(out=out_d[g].rearrange("(o c) -> o c", o=1), in_=prob)


_NC_CACHE = None


def kernel(x, adj, Ws, a1, a2, W1, b1, W2, b2):
    global _NC_CACHE
    if _NC_CACHE is None:
        _NC_CACHE = build()
    nc = _NC_CACHE

    x = np.ascontiguousarray(np.asarray(x, dtype=np.float32))
    adj = np.ascontiguousarray(np.asarray(adj, dtype=np.float32))
    Ws = np.asarray(Ws, np.float32); a1 = np.asarray(a1, np.float32)
    a2 = np.asarray(a2, np.float32); W1 = np.asarray(W1, np.float32)
    b1 = np.asarray(b1, np.float32); W2 = np.asarray(W2, np.float32)
    b2 = np.asarray(b2, np.float32)
    wpack = np.zeros((128, WCOLS), np.float32)
    wpack[:, 0:192] = Ws.transpose(1, 0, 2).reshape(128, L * H)
    wpack[0:64, 192:576] = W1.reshape(L, H, FC).transpose(1, 0, 2).reshape(H, L * FC)
    wpack[:, 576:586] = W2
    wa1 = np.einsum("lfh,lh->lf", Ws, a1)           # [L, F]
    wa2 = np.einsum("lfh,lh->lf", Ws, a2)           # [L, F]
    wpack[:, 586:589] = wa2.T
    wpack[:, 589:973] = np.repeat(wa1[:, :, None], 128, axis=2).transpose(
        1, 0, 2).reshape(128, L * 128)
    wpack[:, 973] = b1
    wpack[0, 974:984] = b2
    shared = {"wpack": wpack}
    in_maps = []
    for k in range(NCORES):
        m = {"x": np.ascontiguousarray(x[k * G:(k + 1) * G]),
             "adj": np.ascontiguousarray(adj[k * G:(k + 1) * G])}
        m.update(shared)
        in_maps.append(m)

    res = run_bass_kernel_spmd(nc, in_maps, core_ids=list(range(NCORES)))
    return np.concatenate([res.results[k]["out"] for k in range(NCORES)],
                          axis=0)


# revision 33
# speedup vs baseline: 1.0372x; 1.0372x over previous
"""GAT (3 parallel attention heads + FC classifier) on 8 Trainium2 NeuronCores.

Sharding: data-parallel over the batch (graph) dim — 2 graphs per core, layer
weights replicated. One SPMD Bass program; per-core inputs are the graph
slices.

Math (per graph, per layer l):
    h = x @ W_l                                  [N, H]
    f1 = h @ a1_l, f2 = h @ a2_l                 [N]
    e_ij = leaky_relu(f1_i + f2_j, 0.2)
    att = softmax_j(where(adj_ij > 0, e, -inf))
    out = relu(att @ h)
Key identity used on-device (lrelu -> max of two exponentials):
    exp(lrelu(t)) = max(exp(t), exp(0.2 t)),  t = f1_i + f2_j
so with p=exp(f1), r=exp(0.2 f1) (free-dim broadcast tiles) and q=exp(f2),
s=exp(0.2 f2) (per-partition scalars), the masked unnormalized attention is
    T_ji = adj_ij * max(p_i * q_j, r_i * s_j)
built in transposed (neighbor-j on partitions) layout so the TensorE can
contract over j directly:  numer/denom = T^T @ [h | 1].
Then out_i = relu(numer_i)/denom_i, mean-pool over nodes folded into the
pooling matmul (rhs = 1/denom column), then FC1+relu, FC2, softmax.
"""
import numpy as np

import concourse.bass as bass
from concourse.masks import make_identity
import concourse.tile as tile
import concourse.mybir as mybir
from concourse.bass_utils import run_bass_kernel_spmd
from concourse.tile_rust import add_dep_helper

f32 = mybir.dt.float32
bf16 = mybir.dt.bfloat16
ALU = mybir.AluOpType
ACTF = mybir.ActivationFunctionType

B, N, F, H, L = 16, 1024, 128, 64, 3
FC, C = 128, 10
NCORES = 8
G = B // NCORES          # graphs per core
NCH = N // 128           # 8 node chunks
LEAK = 0.2
NBF = 973                # bf16-cast columns of the packed weight tile
WCOLS = 984


def _split_multi_waits(nc):
    """The cayman ISA structs have exactly one embedded sync-wait slot and
    this walrus build refuses instructions with more; split extras into
    preceding single-wait NoOp carriers on the same engine."""
    n = 0
    for fn in nc.m.functions:
        for blk in fn.blocks:
            out = []
            for inst in blk.instructions:
                si = inst.sync_info
                if si is not None and si.on_wait and len(si.on_wait) > 1:
                    waits = list(si.on_wait)
                    for w in waits[1:]:
                        out.append(mybir.InstNoOp(
                            name=f"{inst.name}_wc{n}", ins=[], outs=[],
                            engine=inst.engine,
                            sync_info=mybir.SyncInfo(on_wait=[w], on_update=[]),
                            bass_nofuse=True))
                        n += 1
                    si.on_wait = waits[:1]
                out.append(inst)
            blk.instructions = out
    return n


def build():
    nc = bass.Bass()

    x_d = nc.dram_tensor("x", [G, N, F], f32, kind="ExternalInput")
    adj_d = nc.dram_tensor("adj", [G, N, N], f32, kind="ExternalInput")
    wp_d = nc.dram_tensor("wpack", [128, WCOLS], f32, kind="ExternalInput")
    out_d = nc.dram_tensor("out", [G, C], f32, kind="ExternalOutput")

    # DRAM scratch: bf16 copies so the 2-byte-only xbar DMA-transpose applies.
    # One tensor per (graph, column-block): Tile tracks DRAM deps per-tensor,
    # so separate tensors keep the cast->transpose pipeline deps exact.
    adjb_scr = [[nc.dram_tensor(f"adjb_{g}_{b}", [N, 256], bf16, kind="Internal")
                 for b in range(4)] for g in range(G)]
    xb_scr = [nc.dram_tensor(f"xb_{g}", [N, F], bf16, kind="Internal")
              for g in range(G)]

    with tile.TileContext(nc) as tc:
        _build_body(nc, tc, x_d, adj_d, wp_d, out_d, adjb_scr, xb_scr)
    _split_multi_waits(nc)
    return nc


def _build_body(nc, tc, x_d, adj_d, wp_d, out_d, adjb_scr, xb_scr):
    from contextlib import ExitStack
    with ExitStack() as ctx:
        ep = ctx.enter_context

        consts = ep(tc.tile_pool(name="consts", bufs=1))
        p_adjT = ep(tc.tile_pool(name="adjT", bufs=2))
        p_xT = ep(tc.tile_pool(name="xT", bufs=2))
        p_bc = ep(tc.tile_pool(name="bc", bufs=3))      # p_b / r_b broadcasts
        p_h = ep(tc.tile_pool(name="h", bufs=3))        # haug / hTb / scaled
        p_w = ep(tc.tile_pool(name="w", bufs=4))        # v / m / T work tiles
        p_sm = ep(tc.tile_pool(name="sm", bufs=3))      # small vectors
        # PSUM: out accumulators 2x1 bank; wide 2x2 banks; small 2x1 bank
        ps_out = ep(tc.tile_pool(name="ps_out", bufs=4, space="PSUM"))
        ps_wide = ep(tc.tile_pool(name="ps_wide", bufs=1, space="PSUM"))
        ps_sm = ep(tc.tile_pool(name="ps_sm", bufs=2, space="PSUM"))
        ps_fc = ep(tc.tile_pool(name="ps_fc", bufs=1, space="PSUM"))

        # ---- identity for PE transposes, generated on-chip early ----
        ident = consts.tile([128, 128], f32)
        with tc.high_priority():
            make_identity(nc, ident)
        # ---- packed weights (host-packed, see kernel()) ----
        wp32 = consts.tile([128, WCOLS], f32)
        nc.sync.dma_start(out=wp32, in_=wp_d[:, :])
        wpb = consts.tile([128, NBF], bf16)
        nc.scalar.copy(wpb, wp32[:, 0:NBF])
        wb = wpb[:, 0:192].rearrange("p (l h) -> p l h", l=L)
        w1b = wpb[0:64, 192:576].rearrange("p (l f) -> p l f", l=L)
        w2b = wpb[:, 576:586]
        wa2c = wpb[:, 586:589]
        wa1m = wpb[:, 589:973].rearrange("p (l m) -> p l m", l=L)
        b1c = wp32[:, 973:974]
        b2r = wp32[0:1, 974:984]

        last_xpose = None
        xTs = []
        for g in range(G):
            # x: load natural, transpose on PE, cast on ACT (both graphs up
            # front: cheap, and the PE is idle during the adj startup)
            from contextlib import nullcontext
            with tc.high_priority() if g == 0 else nullcontext():
                x_sb = p_xT.tile([128, NCH, F], f32, tag="x_sb")
                nc.sync.dma_start(
                    out=x_sb, in_=x_d[g].rearrange("(c p) f -> p c f", p=128))
                xT = p_xT.tile([F, N], bf16, tag="xT")    # [feat, node]
                for half in range(2):
                    xt_ps = ps_wide.tile([128, 512], f32, tag="wide")
                    for cc in range(4):
                        nc.tensor.transpose(xt_ps[:, cc * 128:(cc + 1) * 128],
                                            x_sb[:, 4 * half + cc, :], ident)
                    nc.scalar.copy(xT[:, half * 512:half * 512 + 512], xt_ps)
            xTs.append(xT)
        for g in range(G):
            xT = xTs[g]
            adjT = p_adjT.tile([128, NCH, N], bf16)   # [j%128, j//128, i]
            for blk in range(4):
                ci = nc.gpsimd.dma_start(out=adjb_scr[g][blk][:, :],
                                         in_=adj_d[g, :, blk * 256:(blk + 1) * 256])
                if g > 0 and last_xpose is not None:
                    # keep graph-1 casts off the DMA fabric until graph-0's
                    # transposes (critical path) are through
                    add_dep_helper(ci.ins, last_xpose.ins, sync=True,
                                   reason="stagger g1 casts")
                for h2 in range(2):
                    xp = nc.sync.dma_start_transpose(
                        out=adjT[:, 2 * blk + h2, :],
                        in_=adjb_scr[g][blk][:, h2 * 128:(h2 + 1) * 128])
            last_xpose = xp

            pcol_ps = ps_fc.tile([H, L], f32, tag="fc")

            def prep(l):
                # f1 broadcast straight from xT: f1 = (W a1) . x, with the
                # W a1 product host-packed replicated as wa1m. The
                # r=exp(LEAK*f1) factor is row-constant and cancels in the
                # softmax ratio, so max(p q, r s) -> r * max(z q, s), r dropped
                z_bt = p_bc.tile([128, N], bf16, tag="z_b")
                for k in range(2):
                    f1bc_ps = ps_wide.tile([128, 512], f32, tag="wide")
                    nc.tensor.matmul(f1bc_ps, wa1m[:, l, :],
                                     xT[:, k * 512:(k + 1) * 512],
                                     start=True, stop=True)
                    nc.scalar.activation(z_bt[:, k * 512:(k + 1) * 512],
                                         f1bc_ps, ACTF.Exp, scale=1.0 - LEAK)
                # f2 per-partition columns: f2 = (W a2) . x
                f2c_ps = ps_sm.tile([128, NCH], f32, tag="small")
                for c in range(NCH):
                    nc.tensor.matmul(f2c_ps[:, c:c + 1],
                                     xT[:, c * 128:(c + 1) * 128],
                                     wa2c[:, l:l + 1], start=True, stop=True)
                q_all = p_sm.tile([128, NCH], f32, tag="q_all")
                nc.scalar.activation(q_all, f2c_ps, ACTF.Exp)
                s_all = p_sm.tile([128, NCH], f32, tag="s_all")
                nc.scalar.activation(s_all, f2c_ps, ACTF.Exp, scale=LEAK)
                # h natural chunks -> haug = [h | 1] (bf16)
                h_ps = ps_sm.tile([128, NCH, H], f32, tag="small")
                for c in range(NCH):
                    nc.tensor.matmul(h_ps[:, c, :],
                                     xT[:, c * 128:(c + 1) * 128], wb[:, l, :],
                                     start=True, stop=True)
                haug = p_h.tile([128, NCH, H + 1], bf16, tag="haug")
                nc.scalar.copy(haug[:, :, 0:H], h_ps)
                nc.vector.memset(haug[:, :, H:H + 1], 1.0)
                oA = ps_out.tile([128, 4, H + 1], f32, tag="out")
                oB = ps_out.tile([128, 4, H + 1], f32, tag="out")
                return z_bt, q_all, s_all, haug, oA, oB

            def chunk2(st, c0, first, last):
                # pair of j-chunks: two gm ops (per-chunk scalars), ONE
                # tensor_tensor mask-multiply over [128, 2048] to amortize
                # the per-op init bubble and halve the DVE DRAIN count
                z_bt, q_all, s_all, haug, oA, oB = st
                gm2 = p_w.tile([128, 2, N], bf16, tag="gm2")
                for k in range(2):
                    c = c0 + k
                    nc.vector.tensor_scalar(
                        out=gm2[:, k, :], in0=z_bt, scalar1=q_all[:, c:c + 1],
                        scalar2=s_all[:, c:c + 1], op0=ALU.mult, op1=ALU.max)
                t2 = p_w.tile([128, 2, N], bf16, tag="t2")
                nc.vector.tensor_mul(t2, gm2, adjT[:, c0:c0 + 2, :])
                for k in range(2):
                    for ib in range(NCH):
                        o = oA if ib < 4 else oB
                        nc.tensor.matmul(o[:, ib % 4, :],
                                         t2[:, k, ib * 128:(ib + 1) * 128],
                                         haug[:, c0 + k, :],
                                         start=first and k == 0,
                                         stop=last and k == 1)

            def post(st, l):
                _, _, _, _, oA, oB = st
                rcp = p_sm.tile([128, NCH], f32, tag="rcp")
                nc.vector.reciprocal(rcp[:, 0:4], oA[:, :, H])
                nc.vector.reciprocal(rcp[:, 4:8], oB[:, :, H])
                rcpb = p_sm.tile([128, NCH], bf16, tag="rcpb")
                nc.scalar.copy(rcpb, rcp)
                scaled = p_h.tile([128, NCH, H], bf16, tag="scaled")
                nc.scalar.activation(scaled[:, 0:4, :], oA[:, :, 0:H], ACTF.Relu)
                nc.scalar.activation(scaled[:, 4:8, :], oB[:, :, 0:H], ACTF.Relu)
                for ib in range(NCH):
                    nc.tensor.matmul(pcol_ps[:, l:l + 1], scaled[:, ib, :],
                                     rcpb[:, ib:ib + 1],
                                     start=(ib == 0), stop=(ib == NCH - 1))

            # layers 0+1 interleaved per chunk (keeps the DVE fed while the
            # adjT transposes stream in), layer 2 after with adjT resident
            if g == 0:
                with tc.high_priority():
                    st0 = prep(0)
            else:
                st0 = prep(0)
            st1 = prep(1)
            for c0 in range(0, NCH, 2):
                chunk2(st0, c0, c0 == 0, c0 == NCH - 2)
                chunk2(st1, c0, c0 == 0, c0 == NCH - 2)
            post(st0, 0)
            post(st1, 1)
            st2 = prep(2)
            for c0 in range(0, NCH, 2):
                chunk2(st2, c0, c0 == 0, c0 == NCH - 2)
            post(st2, 2)

            # ---- classifier head ----
            pcolb = p_sm.tile([H, L], bf16, tag="pcolb")
            nc.scalar.copy(pcolb, pcol_ps)
            z_ps = ps_fc.tile([FC, 1], f32, tag="fc")
            for l in range(L):
                nc.tensor.matmul(z_ps, w1b[:, l, :], pcolb[:, l:l + 1],
                                 start=(l == 0), stop=(l == L - 1))
            zr = p_sm.tile([FC, 1], bf16, tag="zr")
            nc.scalar.activation(zr, z_ps, ACTF.Relu, bias=b1c, scale=1.0 / N)
            lg_ps = ps_fc.tile([1, C], f32, tag="fc")
            nc.tensor.matmul(lg_ps, zr, w2b, start=True, stop=True)
            lg = p_sm.tile([1, C], f32, tag="lg")
            nc.vector.tensor_add(lg, lg_ps, b2r)
            mx = p_sm.tile([1, 1], f32, tag="mx")
            nc.vector.reduce_max(mx, lg, axis=mybir.AxisListType.X)
            nmx = p_sm.tile([1, 1], f32, tag="nmx")
            nc.vector.tensor_scalar_mul(nmx, mx, -1.0)
            e = p_sm.tile([1, C], f32, tag="e")
            se = p_sm.tile([1, 1], f32, tag="se")
            nc.scalar.activation(e, lg, ACTF.Exp, bias=nmx, accum_out=se)
            rse = p_sm.tile([1, 1], f32, tag="rse")
            nc.vector.reciprocal(rse, se)
            prob = p_sm.tile([1, C], f32, tag="prob")
            nc.vector.tensor_scalar_mul(prob, e, rse[0:1, 0:1])
            nc.sync.dma_start(out=out_d[g].rearrange("(o c) -> o c", o=1), in_=prob)


_NC_CACHE = None


def kernel(x, adj, Ws, a1, a2, W1, b1, W2, b2):
    global _NC_CACHE
    if _NC_CACHE is None:
        _NC_CACHE = build()
    nc = _NC_CACHE

    x = np.ascontiguousarray(np.asarray(x, dtype=np.float32))
    adj = np.ascontiguousarray(np.asarray(adj, dtype=np.float32))
    Ws = np.asarray(Ws, np.float32); a1 = np.asarray(a1, np.float32)
    a2 = np.asarray(a2, np.float32); W1 = np.asarray(W1, np.float32)
    b1 = np.asarray(b1, np.float32); W2 = np.asarray(W2, np.float32)
    b2 = np.asarray(b2, np.float32)
    wpack = np.zeros((128, WCOLS), np.float32)
    wpack[:, 0:192] = Ws.transpose(1, 0, 2).reshape(128, L * H)
    wpack[0:64, 192:576] = W1.reshape(L, H, FC).transpose(1, 0, 2).reshape(H, L * FC)
    wpack[:, 576:586] = W2
    wa1 = np.einsum("lfh,lh->lf", Ws, a1)           # [L, F]
    wa2 = np.einsum("lfh,lh->lf", Ws, a2)           # [L, F]
    wpack[:, 586:589] = wa2.T
    wpack[:, 589:973] = np.repeat(wa1[:, :, None], 128, axis=2).transpose(
        1, 0, 2).reshape(128, L * 128)
    wpack[:, 973] = b1
    wpack[0, 974:984] = b2
    shared = {"wpack": wpack}
    in_maps = []
    for k in range(NCORES):
        m = {"x": np.ascontiguousarray(x[k * G:(k + 1) * G]),
             "adj": np.ascontiguousarray(adj[k * G:(k + 1) * G])}
        m.update(shared)
        in_maps.append(m)

    res = run_bass_kernel_spmd(nc, in_maps, core_ids=list(range(NCORES)))
    return np.concatenate([res.results[k]["out"] for k in range(NCORES)],
                          axis=0)


# revision 38
# speedup vs baseline: 1.0374x; 1.0002x over previous
"""GAT (3 parallel attention heads + FC classifier) on 8 Trainium2 NeuronCores.

Sharding: data-parallel over the batch (graph) dim — 2 graphs per core, layer
weights replicated. One SPMD Bass program; per-core inputs are the graph
slices.

Math (per graph, per layer l):
    h = x @ W_l                                  [N, H]
    f1 = h @ a1_l, f2 = h @ a2_l                 [N]
    e_ij = leaky_relu(f1_i + f2_j, 0.2)
    att = softmax_j(where(adj_ij > 0, e, -inf))
    out = relu(att @ h)
Key identity used on-device (lrelu -> max of two exponentials):
    exp(lrelu(t)) = max(exp(t), exp(0.2 t)),  t = f1_i + f2_j
so with p=exp(f1), r=exp(0.2 f1) (free-dim broadcast tiles) and q=exp(f2),
s=exp(0.2 f2) (per-partition scalars), the masked unnormalized attention is
    T_ji = adj_ij * max(p_i * q_j, r_i * s_j)
built in transposed (neighbor-j on partitions) layout so the TensorE can
contract over j directly:  numer/denom = T^T @ [h | 1].
Then out_i = relu(numer_i)/denom_i, mean-pool over nodes folded into the
pooling matmul (rhs = 1/denom column), then FC1+relu, FC2, softmax.
"""
import numpy as np

import concourse.bass as bass
from concourse.masks import make_identity
import concourse.tile as tile
import concourse.mybir as mybir
from concourse.bass_utils import run_bass_kernel_spmd
from concourse.tile_rust import add_dep_helper

f32 = mybir.dt.float32
bf16 = mybir.dt.bfloat16
ALU = mybir.AluOpType
ACTF = mybir.ActivationFunctionType

B, N, F, H, L = 16, 1024, 128, 64, 3
FC, C = 128, 10
NCORES = 8
G = B // NCORES          # graphs per core
NCH = N // 128           # 8 node chunks
LEAK = 0.2
NBF = 973                # bf16-cast columns of the packed weight tile
WCOLS = 984


def _split_multi_waits(nc):
    """The cayman ISA structs have exactly one embedded sync-wait slot and
    this walrus build refuses instructions with more; split extras into
    preceding single-wait NoOp carriers on the same engine."""
    n = 0
    for fn in nc.m.functions:
        for blk in fn.blocks:
            out = []
            for inst in blk.instructions:
                si = inst.sync_info
                if si is not None and si.on_wait and len(si.on_wait) > 1:
                    waits = list(si.on_wait)
                    for w in waits[1:]:
                        out.append(mybir.InstNoOp(
                            name=f"{inst.name}_wc{n}", ins=[], outs=[],
                            engine=inst.engine,
                            sync_info=mybir.SyncInfo(on_wait=[w], on_update=[]),
                            bass_nofuse=True))
                        n += 1
                    si.on_wait = waits[:1]
                out.append(inst)
            blk.instructions = out
    return n


def build():
    nc = bass.Bass()

    x_d = nc.dram_tensor("x", [G, N, F], f32, kind="ExternalInput")
    adj_d = nc.dram_tensor("adj", [G, N, N], f32, kind="ExternalInput")
    wp_d = nc.dram_tensor("wpack", [128, WCOLS], f32, kind="ExternalInput")
    out_d = nc.dram_tensor("out", [G, C], f32, kind="ExternalOutput")

    # DRAM scratch: bf16 copies so the 2-byte-only xbar DMA-transpose applies.
    # One tensor per (graph, column-block): Tile tracks DRAM deps per-tensor,
    # so separate tensors keep the cast->transpose pipeline deps exact.
    adjb_scr = [[nc.dram_tensor(f"adjb_{g}_{b}", [N, 256], bf16, kind="Internal")
                 for b in range(4)] for g in range(G)]
    xb_scr = [nc.dram_tensor(f"xb_{g}", [N, F], bf16, kind="Internal")
              for g in range(G)]

    with tile.TileContext(nc) as tc:
        _build_body(nc, tc, x_d, adj_d, wp_d, out_d, adjb_scr, xb_scr)
    _split_multi_waits(nc)
    return nc


def _build_body(nc, tc, x_d, adj_d, wp_d, out_d, adjb_scr, xb_scr):
    from contextlib import ExitStack
    with ExitStack() as ctx:
        ep = ctx.enter_context

        consts = ep(tc.tile_pool(name="consts", bufs=1))
        p_adjT = ep(tc.tile_pool(name="adjT", bufs=2))
        p_xT = ep(tc.tile_pool(name="xT", bufs=2))
        p_bc = ep(tc.tile_pool(name="bc", bufs=3))      # p_b / r_b broadcasts
        p_h = ep(tc.tile_pool(name="h", bufs=3))        # haug / hTb / scaled
        p_w = ep(tc.tile_pool(name="w", bufs=6))        # v / m / T work tiles
        p_sm = ep(tc.tile_pool(name="sm", bufs=3))      # small vectors
        # PSUM: out accumulators 2x1 bank; wide 2x2 banks; small 2x1 bank
        ps_out = ep(tc.tile_pool(name="ps_out", bufs=4, space="PSUM"))
        ps_wide = ep(tc.tile_pool(name="ps_wide", bufs=1, space="PSUM"))
        ps_sm = ep(tc.tile_pool(name="ps_sm", bufs=2, space="PSUM"))
        ps_fc = ep(tc.tile_pool(name="ps_fc", bufs=1, space="PSUM"))

        # ---- identity for PE transposes, generated on-chip early ----
        ident = consts.tile([128, 128], f32)
        with tc.high_priority():
            make_identity(nc, ident)
        # ---- packed weights (host-packed, see kernel()) ----
        wp32 = consts.tile([128, WCOLS], f32)
        nc.sync.dma_start(out=wp32, in_=wp_d[:, :])
        wpb = consts.tile([128, NBF], bf16)
        nc.scalar.copy(wpb, wp32[:, 0:NBF])
        wb = wpb[:, 0:192].rearrange("p (l h) -> p l h", l=L)
        w1b = wpb[0:64, 192:576].rearrange("p (l f) -> p l f", l=L)
        w2b = wpb[:, 576:586]
        wa2c = wpb[:, 586:589]
        wa1m = wpb[:, 589:973].rearrange("p (l m) -> p l m", l=L)
        b1c = wp32[:, 973:974]
        b2r = wp32[0:1, 974:984]

        last_xpose = None
        xTs = []
        for g in range(G):
            # x: load natural, transpose on PE, cast on ACT (both graphs up
            # front: cheap, and the PE is idle during the adj startup)
            from contextlib import nullcontext
            with tc.high_priority() if g == 0 else nullcontext():
                x_sb = p_xT.tile([128, NCH, F], f32, tag="x_sb")
                nc.sync.dma_start(
                    out=x_sb, in_=x_d[g].rearrange("(c p) f -> p c f", p=128))
                xT = p_xT.tile([F, N], bf16, tag="xT")    # [feat, node]
                for half in range(2):
                    xt_ps = ps_wide.tile([128, 512], f32, tag="wide")
                    for cc in range(4):
                        nc.tensor.transpose(xt_ps[:, cc * 128:(cc + 1) * 128],
                                            x_sb[:, 4 * half + cc, :], ident)
                    nc.scalar.copy(xT[:, half * 512:half * 512 + 512], xt_ps)
            xTs.append(xT)
        for g in range(G):
            xT = xTs[g]
            adjT = p_adjT.tile([128, NCH, N], bf16)   # [j%128, j//128, i]
            for blk in range(4):
                ci = nc.gpsimd.dma_start(out=adjb_scr[g][blk][:, :],
                                         in_=adj_d[g, :, blk * 256:(blk + 1) * 256])
                if g > 0 and last_xpose is not None:
                    # keep graph-1 casts off the DMA fabric until graph-0's
                    # transposes (critical path) are through
                    add_dep_helper(ci.ins, last_xpose.ins, sync=True,
                                   reason="stagger g1 casts")
                for h2 in range(2):
                    xp = nc.sync.dma_start_transpose(
                        out=adjT[:, 2 * blk + h2, :],
                        in_=adjb_scr[g][blk][:, h2 * 128:(h2 + 1) * 128])
            last_xpose = xp

            pcol_ps = ps_fc.tile([H, L], f32, tag="fc")

            def prep(l):
                # f1 broadcast straight from xT: f1 = (W a1) . x, with the
                # W a1 product host-packed replicated as wa1m. The
                # r=exp(LEAK*f1) factor is row-constant and cancels in the
                # softmax ratio, so max(p q, r s) -> r * max(z q, s), r dropped
                z_bt = p_bc.tile([128, N], bf16, tag="z_b")
                for k in range(2):
                    f1bc_ps = ps_wide.tile([128, 512], f32, tag="wide")
                    nc.tensor.matmul(f1bc_ps, wa1m[:, l, :],
                                     xT[:, k * 512:(k + 1) * 512],
                                     start=True, stop=True)
                    nc.scalar.activation(z_bt[:, k * 512:(k + 1) * 512],
                                         f1bc_ps, ACTF.Exp, scale=1.0 - LEAK)
                # f2 per-partition columns: f2 = (W a2) . x
                f2c_ps = ps_sm.tile([128, NCH], f32, tag="small")
                for c in range(NCH):
                    nc.tensor.matmul(f2c_ps[:, c:c + 1],
                                     xT[:, c * 128:(c + 1) * 128],
                                     wa2c[:, l:l + 1], start=True, stop=True)
                q_all = p_sm.tile([128, NCH], f32, tag="q_all")
                nc.scalar.activation(q_all, f2c_ps, ACTF.Exp)
                s_all = p_sm.tile([128, NCH], f32, tag="s_all")
                nc.scalar.activation(s_all, f2c_ps, ACTF.Exp, scale=LEAK)
                # h natural chunks -> haug = [h | 1] (bf16)
                h_ps = ps_sm.tile([128, NCH, H], f32, tag="small")
                for c in range(NCH):
                    nc.tensor.matmul(h_ps[:, c, :],
                                     xT[:, c * 128:(c + 1) * 128], wb[:, l, :],
                                     start=True, stop=True)
                haug = p_h.tile([128, NCH, H + 1], bf16, tag="haug")
                nc.scalar.copy(haug[:, :, 0:H], h_ps)
                nc.vector.memset(haug[:, :, H:H + 1], 1.0)
                oA = ps_out.tile([128, 4, H + 1], f32, tag="out")
                oB = ps_out.tile([128, 4, H + 1], f32, tag="out")
                return z_bt, q_all, s_all, haug, oA, oB

            def chunk2(st, c0, first, last):
                # pair of j-chunks: two gm ops (per-chunk scalars), ONE
                # tensor_tensor mask-multiply over [128, 2048] to amortize
                # the per-op init bubble and halve the DVE DRAIN count
                z_bt, q_all, s_all, haug, oA, oB = st
                gm2 = p_w.tile([128, 2, N], bf16, tag="gm2")
                for k in range(2):
                    c = c0 + k
                    nc.vector.tensor_scalar(
                        out=gm2[:, k, :], in0=z_bt, scalar1=q_all[:, c:c + 1],
                        scalar2=s_all[:, c:c + 1], op0=ALU.mult, op1=ALU.max)
                t2 = p_w.tile([128, 2, N], bf16, tag="t2")
                nc.vector.tensor_mul(t2, gm2, adjT[:, c0:c0 + 2, :])
                for k in range(2):
                    for ib in range(NCH):
                        o = oA if ib < 4 else oB
                        nc.tensor.matmul(o[:, ib % 4, :],
                                         t2[:, k, ib * 128:(ib + 1) * 128],
                                         haug[:, c0 + k, :],
                                         start=first and k == 0,
                                         stop=last and k == 1)

            def post(st, l):
                _, _, _, _, oA, oB = st
                rcp = p_sm.tile([128, NCH], f32, tag="rcp")
                nc.vector.reciprocal(rcp[:, 0:4], oA[:, :, H])
                nc.vector.reciprocal(rcp[:, 4:8], oB[:, :, H])
                rcpb = p_sm.tile([128, NCH], bf16, tag="rcpb")
                nc.scalar.copy(rcpb, rcp)
                scaled = p_h.tile([128, NCH, H], bf16, tag="scaled")
                nc.scalar.activation(scaled[:, 0:4, :], oA[:, :, 0:H], ACTF.Relu)
                nc.scalar.activation(scaled[:, 4:8, :], oB[:, :, 0:H], ACTF.Relu)
                for ib in range(NCH):
                    nc.tensor.matmul(pcol_ps[:, l:l + 1], scaled[:, ib, :],
                                     rcpb[:, ib:ib + 1],
                                     start=(ib == 0), stop=(ib == NCH - 1))

            # layers 0+1 interleaved per chunk (keeps the DVE fed while the
            # adjT transposes stream in), layer 2 after with adjT resident
            if g == 0:
                with tc.high_priority():
                    st0 = prep(0)
            else:
                st0 = prep(0)
            st1 = prep(1)
            for c0 in range(0, NCH, 2):
                chunk2(st0, c0, c0 == 0, c0 == NCH - 2)
                chunk2(st1, c0, c0 == 0, c0 == NCH - 2)
            post(st0, 0)
            post(st1, 1)
            st2 = prep(2)
            for c0 in range(0, NCH, 2):
                chunk2(st2, c0, c0 == 0, c0 == NCH - 2)
            post(st2, 2)

            # ---- classifier head ----
            pcolb = p_sm.tile([H, L], bf16, tag="pcolb")
            nc.scalar.copy(pcolb, pcol_ps)
            z_ps = ps_fc.tile([FC, 1], f32, tag="fc")
            for l in range(L):
                nc.tensor.matmul(z_ps, w1b[:, l, :], pcolb[:, l:l + 1],
                                 start=(l == 0), stop=(l == L - 1))
            zr = p_sm.tile([FC, 1], bf16, tag="zr")
            nc.scalar.activation(zr, z_ps, ACTF.Relu, bias=b1c, scale=1.0 / N)
            lg_ps = ps_fc.tile([1, C], f32, tag="fc")
            nc.tensor.matmul(lg_ps, zr, w2b, start=True, stop=True)
            lg = p_sm.tile([1, C], f32, tag="lg")
            nc.vector.tensor_add(lg, lg_ps, b2r)
            mx = p_sm.tile([1, 1], f32, tag="mx")
            nc.vector.reduce_max(mx, lg, axis=mybir.AxisListType.X)
            nmx = p_sm.tile([1, 1], f32, tag="nmx")
            nc.vector.tensor_scalar_mul(nmx, mx, -1.0)
            e = p_sm.tile([1, C], f32, tag="e")
            se = p_sm.tile([1, 1], f32, tag="se")
            nc.scalar.activation(e, lg, ACTF.Exp, bias=nmx, accum_out=se)
            rse = p_sm.tile([1, 1], f32, tag="rse")
            nc.vector.reciprocal(rse, se)
            prob = p_sm.tile([1, C], f32, tag="prob")
            nc.vector.tensor_scalar_mul(prob, e, rse[0:1, 0:1])
            nc.sync.dma_start(out=out_d[g].rearrange("(o c) -> o c", o=1), in_=prob)


_NC_CACHE = None


def kernel(x, adj, Ws, a1, a2, W1, b1, W2, b2):
    global _NC_CACHE
    if _NC_CACHE is None:
        _NC_CACHE = build()
    nc = _NC_CACHE

    x = np.ascontiguousarray(np.asarray(x, dtype=np.float32))
    adj = np.ascontiguousarray(np.asarray(adj, dtype=np.float32))
    Ws = np.asarray(Ws, np.float32); a1 = np.asarray(a1, np.float32)
    a2 = np.asarray(a2, np.float32); W1 = np.asarray(W1, np.float32)
    b1 = np.asarray(b1, np.float32); W2 = np.asarray(W2, np.float32)
    b2 = np.asarray(b2, np.float32)
    wpack = np.zeros((128, WCOLS), np.float32)
    wpack[:, 0:192] = Ws.transpose(1, 0, 2).reshape(128, L * H)
    wpack[0:64, 192:576] = W1.reshape(L, H, FC).transpose(1, 0, 2).reshape(H, L * FC)
    wpack[:, 576:586] = W2
    wa1 = np.einsum("lfh,lh->lf", Ws, a1)           # [L, F]
    wa2 = np.einsum("lfh,lh->lf", Ws, a2)           # [L, F]
    wpack[:, 586:589] = wa2.T
    wpack[:, 589:973] = np.repeat(wa1[:, :, None], 128, axis=2).transpose(
        1, 0, 2).reshape(128, L * 128)
    wpack[:, 973] = b1
    wpack[0, 974:984] = b2
    shared = {"wpack": wpack}
    in_maps = []
    for k in range(NCORES):
        m = {"x": np.ascontiguousarray(x[k * G:(k + 1) * G]),
             "adj": np.ascontiguousarray(adj[k * G:(k + 1) * G])}
        m.update(shared)
        in_maps.append(m)

    res = run_bass_kernel_spmd(nc, in_maps, core_ids=list(range(NCORES)))
    return np.concatenate([res.results[k]["out"] for k in range(NCORES)],
                          axis=0)
